# revision 3
# baseline (speedup 1.0000x reference)
"""BinaryLinear TRN2 kernel: z = x @ sign(weight).T + bias.

x [8192, 4096] f32, weight [4096, 4096] f32, bias [4096] f32 (zeros).

Strategy (8 NeuronCores, SPMD, no collectives):
  - Data-parallel over the 8192-token batch dim: core c computes rows
    c*1024..(c+1)*1024 of z. weight is replicated to every core.
  - The matmul runs in fp8 (e4m3) DoubleRow mode: each InstMatmult
    contracts TWO 128-deep k-planes at 0.5 cycles per output row - 2x
    the f32r/bf16 MAC rate. Precision is recovered with a two-pass
    hi/lo split of the activations:
        x2 = 2*x (host),  hi = Q8(x2),  lo = Q8(x2 - hi)
        z  = (hi + lo) @ (sign(w)/2).T        (exact +-0.5 weights)
    giving ~1.8e-3 relative error (fp8 split 6.5e-4 + bf16 z out).
  - No PE transposes at all: the host ships x pre-transposed
    (2*x_shard.T as f32, plane-major [32,128,1024]) and the weight
    pre-transposed/bf16 ([128 kp, 32 plane, 4096 out], replicated).
    On device the weight is binarized to +-0.5 fp8 with a single
    DVE/Pool tensor_scalar (is_ge then subtract 0.5) and x is
    quantized hi (ACT cast) / lo (DVE subtract) - the PE runs ONLY
    the 4096 DoubleRow matmuls (~524k cycles/core).
  - Weight blocks of 256 out-features are double-buffered and prepped
    one block ahead; x-plane pairs are separate tiles so the first
    block's matmuls chase the x-quantize stream instead of waiting
    for all of phase 1.
  - Output is written as zT [4096, 1024] bf16; the host transposes and
    upcasts on gather.
"""

import numpy as np
import ml_dtypes

import concourse.bacc as bacc
import concourse.bass as bass
import concourse.mybir as mybir
import concourse.tile as tile
from concourse import bass_utils
from concourse.bass import ts

P = 128
N_CORES = 8
N_TOK, K_IN, N_OUT = 8192, 4096, 4096
T = N_TOK // N_CORES  # 1024 tokens per core
KT = K_IN // P  # 32 k-planes
NPAIR = KT // 2  # 16 DoubleRow k-pair steps
OB = 256  # out-feature block width
NOB = N_OUT // OB  # 16 blocks
TP = 256  # token chunk per psum tile (moving free = 2*256 = 512, max)
NTP = T // TP  # 4
BCH = 4  # k-planes per binarize op

F32 = mybir.dt.float32
BF16 = mybir.dt.bfloat16
F8 = mybir.dt.float8e4

_cached_nc = None


def _build_program(loop: int = 0):
    """loop=0: plain kernel. loop=L>0: body wrapped in an on-device For_i
    (used for HW timing via the slope method)."""
    nc = bacc.Bacc("TRN2", target_bir_lowering=False, debug=False)
    # 2*x shard, transposed, plane-major: [plane, kp, tok]
    x2_d = nc.dram_tensor("x2", [KT, P, T], F32, kind="ExternalInput")
    # w transposed, kp-major: [kp, plane, out] bf16
    w_d = nc.dram_tensor("wt", [P, KT, N_OUT], BF16, kind="ExternalInput")
    zs_d = nc.dram_tensor("zs", [N_OUT, T], BF16, kind="ExternalOutput")

    import contextlib

    DR = mybir.MatmulPerfMode.DoubleRow
    IS_GE = mybir.AluOpType.is_ge
    SUB = mybir.AluOpType.subtract

    with tile.TileContext(nc) as tc:
        with (
            tc.tile_pool(name="xraw", bufs=4) as xrawp,
            tc.tile_pool(name="xhi", bufs=1) as xhip,
            tc.tile_pool(name="xlo", bufs=1) as xlop,
            tc.tile_pool(name="wraw", bufs=2) as wrawp,
            tc.tile_pool(name="wbin", bufs=2) as wbinp,
            tc.tile_pool(name="zst", bufs=3) as zstp,
            tc.tile_pool(name="psm", bufs=8, space="PSUM") as psmp,
        ):
            # persistent fp8 x tiles, one per DoubleRow k-pair
            xh = [xhip.tile([P, 2, T], F8, name=f"xh{i}") for i in range(NPAIR)]
            xl = [xlop.tile([P, 2, T], F8, name=f"xl{i}") for i in range(NPAIR)]

            loop_cm = tc.For_i(0, loop, 1) if loop else contextlib.nullcontext()
            with loop_cm:
                # ---- Phase 1 (streams; matmuls chase it): quantize x ----
                for pl in range(KT):
                    xr = xrawp.tile([P, T], F32, name="xr", tag="xr")
                    nc.sync.dma_start(xr[:], x2_d.ap()[pl])
                    pair, half = divmod(pl, 2)
                    nc.scalar.copy(xh[pair][:, half, :], xr[:])
                    nc.vector.tensor_tensor(
                        xl[pair][:, half, :], xr[:], xh[pair][:, half, :], SUB
                    )

                # ---- Weight block prep: DMA bf16 wT block, binarize to
                # +-0.5 fp8 on DVE/Pool (alternating) ----
                def prep(ob):
                    wr = wrawp.tile([P, KT, OB], BF16, name="wr", tag="wr")
                    nc.sync.dma_start(wr[:], w_d.ap()[:, :, ts(ob, OB)])
                    wb = wbinp.tile([P, KT, OB], F8, name="wb", tag="wb")
                    for g in range(KT // BCH):
                        eng = nc.vector if g % 2 == 0 else nc.gpsimd
                        eng.tensor_scalar(
                            wb[:, ts(g, BCH), :],
                            wr[:, ts(g, BCH), :],
                            0.0,
                            0.5,
                            IS_GE,
                            SUB,
                        )
                    return wb

                def mm_block(ob, wb):
                    # zT orientation: psum tiles [128 out, 256 tok];
                    # stationary = fp8 +-0.5 weight k-pair (shared by the
                    # 8 streams: {hi,lo} x 4 token chunks)
                    for oi in range(OB // P):
                        pms = [
                            psmp.tile([P, TP], F32, name="pm", tag="pm")
                            for _ in range(NTP)
                        ]
                        for k in range(NPAIR):
                            st = wb[:, 2 * k : 2 * k + 2, ts(oi, P)]
                            for si, src in enumerate((xh, xl)):
                                for tp in range(NTP):
                                    nc.tensor.matmul(
                                        pms[tp][:],
                                        st,
                                        src[k][:, :, ts(tp, TP)],
                                        start=(k == 0 and si == 0),
                                        stop=(k == NPAIR - 1 and si == 1),
                                        perf_mode=DR,
                                    )
                        zt = zstp.tile([P, T], BF16, name="zt", tag="zt")
                        for tp in range(NTP):
                            nc.scalar.copy(zt[:, ts(tp, TP)], pms[tp][:])
                        nc.sync.dma_start(zs_d.ap()[ts(ob * 2 + oi, P), :], zt[:])

                wb_cur = prep(0)
                for ob in range(NOB):
                    wb_next = prep(ob + 1) if ob + 1 < NOB else None
                    mm_block(ob, wb_cur)
                    wb_cur = wb_next
    nc.compile()
    return nc


def _get_nc():
    global _cached_nc
    if _cached_nc is None:
        _cached_nc = _build_program()
    return _cached_nc


def _host_inputs(x, weight):
    """Per-core input dicts: 2*x shard transposed (f32, plane-major) and
    the transposed bf16 weight (replicated)."""
    x2 = (2.0 * x).astype(np.float32)
    wt = np.ascontiguousarray(
        weight.T.reshape(KT, P, N_OUT).transpose(1, 0, 2)
    ).astype(ml_dtypes.bfloat16)
    in_maps = []
    for c in range(N_CORES):
        x2t = np.ascontiguousarray(x2[c * T : (c + 1) * T].T).reshape(KT, P, T)
        in_maps.append({"x2": x2t, "wt": wt})
    return in_maps


def kernel(x: np.ndarray, weight: np.ndarray, bias: np.ndarray) -> np.ndarray:
    x = np.ascontiguousarray(np.asarray(x, dtype=np.float32))
    weight = np.ascontiguousarray(np.asarray(weight, dtype=np.float32))
    bias = np.ascontiguousarray(np.asarray(bias, dtype=np.float32))
    assert x.shape == (N_TOK, K_IN) and weight.shape == (N_OUT, K_IN)

    nc = _get_nc()
    in_maps = _host_inputs(x, weight)
    res = bass_utils.run_bass_kernel_spmd(nc, in_maps, core_ids=list(range(N_CORES)))
    z = np.empty((N_TOK, N_OUT), dtype=np.float32)
    for c in range(N_CORES):
        np.copyto(z[c * T : (c + 1) * T], res.results[c]["zs"].T.astype(np.float32))
    if np.any(bias):
        z += bias[None, :]
    return z


# ---------------------------------------------------------------------------
# HW timing support (not used by the grading path; test.py calls this).
# The axon PJRT dispatch overhead (~57 ms, noisy) swamps a single kernel
# execution, so we measure on-device time with a For_i-looped variant:
# slope of min wall time vs loop count. Dispatch noise is additive-positive,
# so the global min per L over all rounds is the robust estimator.
# ---------------------------------------------------------------------------


def _make_runner(nc, n_cores=N_CORES):
    import jax
    from jax.sharding import Mesh, PartitionSpec
    from jax.experimental.shard_map import shard_map
    from concourse import bass2jax

    bass2jax.install_neuronx_cc_hook()
    partition_name = nc.partition_id_tensor.name if nc.partition_id_tensor else None
    in_names, out_names, out_avals, zero_outs = [], [], [], []
    for alloc in nc.m.functions[0].allocations:
        if not isinstance(alloc, mybir.MemoryLocationSet):
            continue
        name = alloc.memorylocations[0].name
        if alloc.kind == "ExternalInput":
            if name != partition_name:
                in_names.append(name)
        elif alloc.kind == "ExternalOutput":
            out_names.append(name)
            out_avals.append(
                jax.core.ShapedArray(tuple(alloc.tensor_shape), mybir.dt.np(alloc.dtype))
            )
            zero_outs.append(
                np.zeros(tuple(alloc.tensor_shape), mybir.dt.np(alloc.dtype))
            )
    n_params, n_outs = len(in_names), len(out_avals)
    all_in_names = list(in_names) + list(out_names)
    if partition_name is not None:
        all_in_names.append(partition_name)

    def _body(*args):
        operands = list(args)
        if partition_name is not None:
            operands.append(bass2jax.partition_id_tensor())
        return tuple(
            bass2jax._bass_exec_p.bind(
                *operands,
                out_avals=tuple(out_avals),
                in_names=tuple(all_in_names),
                out_names=tuple(out_names),
                lowering_input_output_aliases=(),
                sim_require_finite=True,
                sim_require_nnan=True,
                nc=nc,
            )
        )

    donate = tuple(range(n_params, n_params + n_outs))
    devices = jax.devices()[:n_cores]
    mesh = Mesh(np.asarray(devices), ("core",))
    in_specs = (PartitionSpec("core"),) * (n_params + n_outs)
    out_specs = (PartitionSpec("core"),) * n_outs
    jitted = jax.jit(
        shard_map(_body, mesh=mesh, in_specs=in_specs, out_specs=out_specs,
                  check_rep=False),
        donate_argnums=donate,
        keep_unused=True,
    )
    return jitted, in_names, zero_outs


def _min_wall(jitted, ins, zero_outs_global, nrep):
    import time
    import jax

    best = float("inf")
    for _ in range(nrep):
        zo = [jax.device_put(z) for z in zero_outs_global]
        jax.block_until_ready(zo)
        t0 = time.perf_counter()
        outs = jitted(*ins, *zo)
        jax.block_until_ready(outs)
        best = min(best, time.perf_counter() - t0)
    return best


def measure_hw_time_ns(inputs, L1=1, L2=33, nrep=8, rounds=4):
    import jax

    x = np.ascontiguousarray(np.asarray(inputs["x"], dtype=np.float32))
    weight = np.ascontiguousarray(np.asarray(inputs["weight"], dtype=np.float32))
    in_maps = _host_inputs(x, weight)

    runners = []
    for L in (L1, L2):
        nc = _build_program(loop=L)
        jitted, in_names, zero_outs = _make_runner(nc)
        concat_in = [
            np.concatenate(
                [np.asarray(in_maps[c][name]) for c in range(N_CORES)], axis=0
            )
            for name in in_names
        ]
        ins = [jax.device_put(a) for a in concat_in]
        jax.block_until_ready(ins)
        zo_global = [np.concatenate([z] * N_CORES, axis=0) for z in zero_outs]
        # warmup
        outs = jitted(*ins, *[jax.device_put(z) for z in zo_global])
        jax.block_until_ready(outs)
        runners.append((jitted, ins, zo_global))

    t_min = [float("inf"), float("inf")]
    for r in range(rounds):
        for i in (0, 1):
            jitted, ins, zo = runners[i]
            t_min[i] = min(t_min[i], _min_wall(jitted, ins, zo, nrep))
        print(
            f"  timing round {r}: t({L1})={t_min[0]*1e3:.2f}ms "
            f"t({L2})={t_min[1]*1e3:.2f}ms "
            f"slope={(t_min[1]-t_min[0])/(L2-L1)*1e9:.0f}ns"
        )
    return (t_min[1] - t_min[0]) / (L2 - L1) * 1e9


# revision 7
# speedup vs baseline: 1.0042x; 1.0042x over previous
"""BinaryLinear TRN2 kernel: z = x @ sign(weight).T + bias.

x [8192, 4096] f32, weight [4096, 4096] f32, bias [4096] f32 (zeros).

Strategy (8 NeuronCores, SPMD, no collectives):
  - Data-parallel over the 8192-token batch dim: core c computes rows
    c*1024..(c+1)*1024 of z. weight is replicated to every core.
  - The matmul runs in fp8 (e4m3) DoubleRow mode: each InstMatmult
    contracts TWO 128-deep k-planes at ~0.5 cycles per output row.
    Precision is recovered with a two-pass hi/lo split of the
    activations:
        x2 = bf16(2*x) (host),  hi = Q8(x2),  lo = Q8(x2 - hi)
        z  = (hi + lo) @ (sign(w)/2).T        (exact +-0.5 weights)
    giving ~1.3e-3 relative error.
  - No PE transposes at all: the host ships x pre-transposed
    (bf16(2*x_shard.T), plane-major [32,128,1024]) and the weight
    pre-transposed/bf16 ([128 kp, 32 plane, 4096 out], replicated).
    On device the weight is binarized to +-0.5 fp8 with a single
    DVE/Pool tensor_scalar (is_ge then subtract 0.5) and x is
    quantized hi (ACT cast) / lo (DVE subtract) - the PE runs ONLY
    DoubleRow matmuls.
  - LDWEIGHTS amortization: moving operands are 1024 wide (psum
    [128,512] = one full bank) and each +-0.5 stationary k-pair is
    shared by 4 streams ({hi,lo} x 2 token halves). tile_legalize
    emits one InstLdweights per matmul even when the stationary is
    unchanged, so a post-pass deletes the redundant loads (DoubleRow
    LDWEIGHTS costs ~2x a normal load - it would otherwise dominate).
  - Weight blocks of 256 out-features are prepped in 8-plane sub-
    chunks, double-buffered one block ahead; x-plane pairs are
    separate tiles so the first block's matmuls chase the x-quantize
    stream instead of waiting for all of phase 1.
  - Output is written as zT [4096, 1024] f32; the host transposes on
    gather.
"""

import numpy as np
import ml_dtypes

import concourse.bacc as bacc
import concourse.bass as bass
import concourse.mybir as mybir
import concourse.tile as tile
from concourse import bass_utils
from concourse.bass import ts

P = 128
N_CORES = 8
N_TOK, K_IN, N_OUT = 8192, 4096, 4096
T = N_TOK // N_CORES  # 1024 tokens per core
KT = K_IN // P  # 32 k-planes
NPAIR = KT // 2  # 16 DoubleRow k-pair steps
OB = 256  # out-feature block width
NOB = N_OUT // OB  # 16 blocks
TH = 512  # token chunk per psum tile (moving free = 2*512 = 1024)
NTH = T // TH  # 2
WCH = 8  # k-planes per w prep sub-chunk
NWCH = KT // WCH  # 4 sub-chunks per block

F32 = mybir.dt.float32
BF16 = mybir.dt.bfloat16
F8 = mybir.dt.float8e4

_cached_nc = None


def _dedupe_ldweights(nc):
    """Remove InstLdweights whose weight AP is identical to the previous
    one with only InstMatmult instructions in between (the PE array still
    holds those weights). Waits on a removed load are hoisted onto the
    next PE instruction."""
    n_removed = 0
    for blk in nc.m.functions[0].blocks:
        insts = blk.instructions
        keep = []
        last_ld_key = None
        pending_waits = []
        for inst in insts:
            if inst.engine != mybir.EngineType.PE:
                keep.append(inst)
                continue
            if isinstance(inst, mybir.InstLdweights):
                key = str(inst.ins[0]) + f"|{inst.perf_mode}"
                if key == last_ld_key:
                    si = inst.sync_info
                    if si is not None:
                        pending_waits.extend(si.on_wait)
                        assert not si.on_update, "dedupe: LD carries updates"
                    n_removed += 1
                    continue
                last_ld_key = key
            elif not isinstance(inst, mybir.InstMatmult):
                # any other PE instruction: stop the sharing run
                last_ld_key = None
            if pending_waits:
                si = inst.sync_info
                if si is None:
                    inst.sync_info = mybir.SyncInfo(
                        on_wait=pending_waits, on_update=[]
                    )
                else:
                    si.on_wait = list(si.on_wait) + pending_waits
                pending_waits = []
            keep.append(inst)
        assert not pending_waits
        blk.instructions = keep
    return n_removed


def _build_program(loop: int = 0):
    """loop=0: plain kernel. loop=L>0: body wrapped in an on-device For_i
    (used for HW timing via the slope method)."""
    nc = bacc.Bacc("TRN2", target_bir_lowering=False, debug=False)
    # bf16(2*x) shard, transposed, plane-major: [plane, kp, tok]
    x2_d = nc.dram_tensor("x2", [KT, P, T], BF16, kind="ExternalInput")
    # w transposed, kp-major: [kp, plane, out] bf16
    w_d = nc.dram_tensor("wt", [P, KT, N_OUT], BF16, kind="ExternalInput")
    zs_d = nc.dram_tensor("zs", [N_OUT, T], F32, kind="ExternalOutput")

    import contextlib

    DR = mybir.MatmulPerfMode.DoubleRow
    IS_GE = mybir.AluOpType.is_ge
    SUB = mybir.AluOpType.subtract

    with tile.TileContext(nc) as tc:
        with (
            tc.tile_pool(name="xraw", bufs=4) as xrawp,
            tc.tile_pool(name="xhi", bufs=1) as xhip,
            tc.tile_pool(name="xlo", bufs=1) as xlop,
            tc.tile_pool(name="wraw", bufs=8) as wrawp,
            tc.tile_pool(name="wbin", bufs=3) as wbinp,
            tc.tile_pool(name="zst", bufs=3) as zstp,
            tc.tile_pool(name="psm", bufs=6, space="PSUM") as psmp,
        ):
            # persistent fp8 x tiles, one per DoubleRow k-pair
            xh = [xhip.tile([P, 2, T], F8, name=f"xh{i}") for i in range(NPAIR)]
            xl = [xlop.tile([P, 2, T], F8, name=f"xl{i}") for i in range(NPAIR)]

            loop_cm = tc.For_i(0, loop, 1) if loop else contextlib.nullcontext()
            with loop_cm:
                # ---- Weight block prep: DMA bf16 wT block in 8-plane
                # sub-chunks, binarize each to +-0.5 fp8 on DVE/Pool ----
                def prep(ob, eng_off=0):
                    subs = []
                    for s in range(NWCH):
                        wr = wrawp.tile([P, WCH, OB], BF16, name="wr", tag="wr")
                        nc.sync.dma_start(
                            wr[:], w_d.ap()[:, s * WCH : (s + 1) * WCH, ts(ob, OB)]
                        )
                        wb = wbinp.tile([P, WCH, OB], F8, name=f"wb{s}", tag=f"wb{s}")
                        eng = nc.vector if (s + eng_off) % 2 == 0 else nc.gpsimd
                        eng.tensor_scalar(wb[:], wr[:], 0.0, 0.5, IS_GE, SUB)
                        subs.append(wb)
                    return subs

                def mm_block(ob, subs):
                    # zT orientation: psum tiles [128 out, 512 tok];
                    # stationary = fp8 +-0.5 weight k-pair, shared by the
                    # 4 streams ({hi,lo} x 2 token halves); redundant
                    # LDWEIGHTS removed by _dedupe_ldweights.
                    for oi in range(OB // P):
                        pms = [
                            psmp.tile([P, TH], F32, name="pm", tag="pm")
                            for _ in range(NTH)
                        ]
                        for k in range(NPAIR):
                            wb = subs[k // (WCH // 2)]
                            kk = k % (WCH // 2)
                            st = wb[:, 2 * kk : 2 * kk + 2, ts(oi, P)]
                            for si, src in enumerate((xh, xl)):
                                for th in range(NTH):
                                    nc.tensor.matmul(
                                        pms[th][:],
                                        st,
                                        src[k][:, :, ts(th, TH)],
                                        start=(k == 0 and si == 0),
                                        stop=(k == NPAIR - 1 and si == 1),
                                        perf_mode=DR,
                                    )
                        zt = zstp.tile([P, T], F32, name="zt", tag="zt")
                        for th in range(NTH):
                            nc.scalar.copy(zt[:, ts(th, TH)], pms[th][:])
                        nc.sync.dma_start(zs_d.ap()[ts(ob * 2 + oi, P), :], zt[:])

                # first two weight blocks are prepped before the x stream so
                # their DMAs land early
                wb_cur = prep(0, 0)
                wb_next = prep(1, 1)

                # ---- Phase 1 (streams; matmuls chase it): quantize x ----
                for pl in range(KT):
                    xr = xrawp.tile([P, T], BF16, name="xr", tag="xr")
                    nc.sync.dma_start(xr[:], x2_d.ap()[pl])
                    pair, half = divmod(pl, 2)
                    nc.scalar.copy(xh[pair][:, half, :], xr[:])
                    nc.vector.tensor_tensor(
                        xl[pair][:, half, :], xr[:], xh[pair][:, half, :], SUB
                    )

                for ob in range(NOB):
                    mm_block(ob, wb_cur)
                    wb_cur = wb_next
                    wb_next = prep(ob + 2, ob) if ob + 2 < NOB else None
    n = _dedupe_ldweights(nc)
    assert 1300 <= n <= 3 * NOB * (OB // P) * NPAIR, n
    nc.compile()
    return nc


def _get_nc():
    global _cached_nc
    if _cached_nc is None:
        _cached_nc = _build_program()
    return _cached_nc


def _host_inputs(x, weight):
    """Per-core input dicts: bf16(2*x) shard transposed (plane-major) and
    the transposed bf16 weight (replicated)."""
    x2 = (2.0 * x).astype(np.float32)
    wt = np.ascontiguousarray(
        weight.T.reshape(KT, P, N_OUT).transpose(1, 0, 2)
    ).astype(ml_dtypes.bfloat16)
    in_maps = []
    for c in range(N_CORES):
        x2t = (
            np.ascontiguousarray(x2[c * T : (c + 1) * T].T)
            .reshape(KT, P, T)
            .astype(ml_dtypes.bfloat16)
        )
        in_maps.append({"x2": x2t, "wt": wt})
    return in_maps


def kernel(x: np.ndarray, weight: np.ndarray, bias: np.ndarray) -> np.ndarray:
    x = np.ascontiguousarray(np.asarray(x, dtype=np.float32))
    weight = np.ascontiguousarray(np.asarray(weight, dtype=np.float32))
    bias = np.ascontiguousarray(np.asarray(bias, dtype=np.float32))
    assert x.shape == (N_TOK, K_IN) and weight.shape == (N_OUT, K_IN)

    nc = _get_nc()
    in_maps = _host_inputs(x, weight)
    res = bass_utils.run_bass_kernel_spmd(nc, in_maps, core_ids=list(range(N_CORES)))
    z = np.empty((N_TOK, N_OUT), dtype=np.float32)
    for c in range(N_CORES):
        np.copyto(z[c * T : (c + 1) * T], res.results[c]["zs"].T)
    if np.any(bias):
        z += bias[None, :]
    return z


# ---------------------------------------------------------------------------
# HW timing support (not used by the grading path; test.py calls this).
# The axon PJRT dispatch overhead (~57-110 ms, noisy) swamps a single kernel
# execution, so we measure on-device time with For_i-looped variants:
# slope of min wall time vs loop count. Dispatch noise is additive-positive,
# so the global min per L over all rounds is the robust estimator; three L
# values let us check linearity.
# ---------------------------------------------------------------------------


def _make_runner(nc, n_cores=N_CORES):
    import jax
    from jax.sharding import Mesh, PartitionSpec
    from jax.experimental.shard_map import shard_map
    from concourse import bass2jax

    bass2jax.install_neuronx_cc_hook()
    partition_name = nc.partition_id_tensor.name if nc.partition_id_tensor else None
    in_names, out_names, out_avals, zero_outs = [], [], [], []
    for alloc in nc.m.functions[0].allocations:
        if not isinstance(alloc, mybir.MemoryLocationSet):
            continue
        name = alloc.memorylocations[0].name
        if alloc.kind == "ExternalInput":
            if name != partition_name:
                in_names.append(name)
        elif alloc.kind == "ExternalOutput":
            out_names.append(name)
            out_avals.append(
                jax.core.ShapedArray(tuple(alloc.tensor_shape), mybir.dt.np(alloc.dtype))
            )
            zero_outs.append(
                np.zeros(tuple(alloc.tensor_shape), mybir.dt.np(alloc.dtype))
            )
    n_params, n_outs = len(in_names), len(out_avals)
    all_in_names = list(in_names) + list(out_names)
    if partition_name is not None:
        all_in_names.append(partition_name)

    def _body(*args):
        operands = list(args)
        if partition_name is not None:
            operands.append(bass2jax.partition_id_tensor())
        return tuple(
            bass2jax._bass_exec_p.bind(
                *operands,
                out_avals=tuple(out_avals),
                in_names=tuple(all_in_names),
                out_names=tuple(out_names),
                lowering_input_output_aliases=(),
                sim_require_finite=True,
                sim_require_nnan=True,
                nc=nc,
            )
        )

    donate = tuple(range(n_params, n_params + n_outs))
    devices = jax.devices()[:n_cores]
    mesh = Mesh(np.asarray(devices), ("core",))
    in_specs = (PartitionSpec("core"),) * (n_params + n_outs)
    out_specs = (PartitionSpec("core"),) * n_outs
    jitted = jax.jit(
        shard_map(_body, mesh=mesh, in_specs=in_specs, out_specs=out_specs,
                  check_rep=False),
        donate_argnums=donate,
        keep_unused=True,
    )
    return jitted, in_names, zero_outs


def _min_wall(jitted, ins, zero_outs_global, nrep):
    import time
    import jax

    best = float("inf")
    for _ in range(nrep):
        zo = [jax.device_put(z) for z in zero_outs_global]
        jax.block_until_ready(zo)
        t0 = time.perf_counter()
        outs = jitted(*ins, *zo)
        jax.block_until_ready(outs)
        best = min(best, time.perf_counter() - t0)
    return best


def measure_hw_time_ns(inputs, Ls=(1, 65, 129), nrep=6, rounds=6):
    import jax

    x = np.ascontiguousarray(np.asarray(inputs["x"], dtype=np.float32))
    weight = np.ascontiguousarray(np.asarray(inputs["weight"], dtype=np.float32))
    in_maps = _host_inputs(x, weight)

    runners = []
    for L in Ls:
        nc = _build_program(loop=L)
        jitted, in_names, zero_outs = _make_runner(nc)
        concat_in = [
            np.concatenate(
                [np.asarray(in_maps[c][name]) for c in range(N_CORES)], axis=0
            )
            for name in in_names
        ]
        ins = [jax.device_put(a) for a in concat_in]
        jax.block_until_ready(ins)
        zo_global = [np.concatenate([z] * N_CORES, axis=0) for z in zero_outs]
        # warmup
        outs = jitted(*ins, *[jax.device_put(z) for z in zo_global])
        jax.block_until_ready(outs)
        runners.append((jitted, ins, zo_global))

    t_min = [float("inf")] * len(Ls)
    for r in range(rounds):
        for i in range(len(Ls)):
            jitted, ins, zo = runners[i]
            t_min[i] = min(t_min[i], _min_wall(jitted, ins, zo, nrep))
        msg = " ".join(f"t({L})={t_min[i]*1e3:.2f}ms" for i, L in enumerate(Ls))
        pair = (t_min[-1] - t_min[0]) / (Ls[-1] - Ls[0]) * 1e9
        print(f"  timing round {r}: {msg} slope={pair:.0f}ns")
    # least-squares slope over the L points
    Lv = np.asarray(Ls, dtype=np.float64)
    tv = np.asarray(t_min, dtype=np.float64)
    slope = float(np.cov(Lv, tv, bias=True)[0, 1] / np.var(Lv))
    return slope * 1e9


# revision 13
# speedup vs baseline: 1.0436x; 1.0392x over previous
"""BinaryLinear TRN2 kernel: z = x @ sign(weight).T + bias.

x [8192, 4096] f32, weight [4096, 4096] f32, bias [4096] f32 (zeros).

Strategy (8 NeuronCores, SPMD, data-parallel over the 8192-token batch;
weight replicated; host does layout/transport prep only, all arithmetic
on device):

  - bf16 single-pass matmul. The PE moving feed is the limit: bf16
    streams 1 column/cycle (16384 MAC/cyc); fp8 DoubleRow doubles that
    but needs a two-pass hi/lo split for precision, landing at the same
    ~437 us/core floor (measured 444-454 us on HW for both). bf16
    single-pass hits the floor with half the instructions, no x
    quantization, FWL-eligible 128-col weight loads, ~1.6e-3 rel err.
  - Host ships bf16(2*x_shard.T) and fp8(w.T) (sign-exact transport,
    half the bytes) in BLOCK-MAJOR layouts so every DMA is long
    contiguous runs per partition (2-8 KB) - transposed-layout 512 B
    runs were the previous bottleneck (~90 GB/s effective; the 33.5 MB
    weight stream dominated the kernel).
  - Device binarize: bitcast the fp8 weight to uint8 and take the sign
    bit: (u8 is_lt 128) - 0.5 = sign(w)/2 with sign(-0)=-1 for
    rounded-up tiny negatives and sign(+0)=+1 - exactly matching the
    reference's sign(0)=+1. One DVE/Pool tensor_scalar per sub-chunk.
    z = (sign(w)/2) @ (2x) exactly.
  - PE runs ONLY matmuls: stationary [128,128] bf16 +-0.5 weights,
    moving 512 tokens, psum [128,512] (one bank), 32-deep accumulation,
    stationary shared across the 2 token chunks (redundant LDWEIGHTS
    deleted post-legalization - the PE array keeps its weights).
  - x streams on the SP DMA queue (prioritized at startup), w blocks on
    the ACT queue in 8-plane sub-chunks binarized ahead (2 blocks in
    flight); z written as bf16 zT, host transposes/upcasts on gather.
"""

import numpy as np
import ml_dtypes

import concourse.bacc as bacc
import concourse.bass as bass
import concourse.mybir as mybir
import concourse.tile as tile
from concourse import bass_utils
from concourse.bass import ts

P = 128
N_CORES = 8
N_TOK, K_IN, N_OUT = 8192, 4096, 4096
T = N_TOK // N_CORES  # 1024 tokens per core
KT = K_IN // P  # 32 k-planes
XG = 4  # x planes per DMA group
NXG = KT // XG  # 8 x tiles
OB = 256  # out-feature block width
NOB = N_OUT // OB  # 16 blocks
TH = 512  # token chunk per psum tile
NTH = T // TH  # 2
WCH = 8  # k-planes per w prep sub-chunk
NWCH = KT // WCH  # 4 sub-chunks per block

F32 = mybir.dt.float32
BF16 = mybir.dt.bfloat16
F8 = mybir.dt.float8e4
U8 = mybir.dt.uint8

_cached_nc = None


def _dedupe_ldweights(nc):
    """Remove InstLdweights whose weight AP is identical to the previous
    one with only InstMatmult instructions in between (the PE array still
    holds those weights). Waits on a removed load are hoisted onto the
    next PE instruction."""
    n_removed = 0
    for blk in nc.m.functions[0].blocks:
        insts = blk.instructions
        keep = []
        last_ld_key = None
        pending_waits = []
        for inst in insts:
            if inst.engine != mybir.EngineType.PE:
                keep.append(inst)
                continue
            if isinstance(inst, mybir.InstLdweights):
                key = str(inst.ins[0]) + f"|{inst.perf_mode}"
                if key == last_ld_key:
                    si = inst.sync_info
                    if si is not None:
                        pending_waits.extend(si.on_wait)
                        assert not si.on_update, "dedupe: LD carries updates"
                    n_removed += 1
                    continue
                last_ld_key = key
            elif not isinstance(inst, mybir.InstMatmult):
                last_ld_key = None
            if pending_waits:
                si = inst.sync_info
                if si is None:
                    inst.sync_info = mybir.SyncInfo(
                        on_wait=pending_waits, on_update=[]
                    )
                else:
                    si.on_wait = list(si.on_wait) + pending_waits
                pending_waits = []
            keep.append(inst)
        assert not pending_waits
        blk.instructions = keep
    return n_removed


def _build_program(loop: int = 0):
    """loop=0: plain kernel. loop=L>0: body wrapped in an on-device For_i
    (used for HW timing via the slope method)."""
    nc = bacc.Bacc("TRN2", target_bir_lowering=False, debug=False)
    # bf16(2*x) shard: [group, kp, plane-in-group, tok] (8 KB runs)
    x2_d = nc.dram_tensor("x2", [NXG, P, XG, T], BF16, kind="ExternalInput")
    # fp8(w.T), block-major: [block, kp, plane, out-in-block] (8 KB runs)
    w_d = nc.dram_tensor("wt", [NOB, P, KT, OB], F8, kind="ExternalInput")
    zs_d = nc.dram_tensor("zs", [N_OUT, T], BF16, kind="ExternalOutput")

    import contextlib

    IS_LT = mybir.AluOpType.is_lt
    SUB = mybir.AluOpType.subtract

    with tile.TileContext(nc) as tc:
        with (
            tc.tile_pool(name="xt", bufs=1) as xtp,
            tc.tile_pool(name="wraw", bufs=8) as wrawp,
            tc.tile_pool(name="wbin", bufs=3) as wbinp,
            tc.tile_pool(name="zst", bufs=3) as zstp,
            tc.tile_pool(name="psm", bufs=6, space="PSUM") as psmp,
        ):
            # resident bf16 x tiles, one per XG-plane group
            xt = [
                xtp.tile([P, XG, T], BF16, name=f"xt{i}") for i in range(NXG)
            ]

            loop_cm = tc.For_i(0, loop, 1) if loop else contextlib.nullcontext()
            with loop_cm:
                # ---- Weight block prep (ACT hwdge queue): DMA fp8 wT
                # sub-chunks (contiguous 2 KB runs), binarize via the uint8
                # sign bit to +-0.5 bf16 on DVE/Pool ----
                def prep(ob, eng_off=0):
                    subs = []
                    for s in range(NWCH):
                        wr = wrawp.tile([P, WCH, OB], F8, name="wr", tag="wr")
                        nc.scalar.dma_start(
                            wr[:], w_d.ap()[ob, :, s * WCH : (s + 1) * WCH, :]
                        )
                        wb = wbinp.tile([P, WCH, OB], BF16, name=f"wb{s}", tag=f"wb{s}")
                        eng = nc.vector if (s + eng_off) % 2 == 0 else nc.gpsimd
                        eng.tensor_scalar(
                            wb[:], wr[:].bitcast(U8), 128.0, 0.5, IS_LT, SUB
                        )
                        subs.append(wb)
                    return subs

                def mm_block(ob, subs):
                    # zT orientation: psum tiles [128 out, 512 tok];
                    # stationary = bf16 +-0.5 weight plane column, shared
                    # by the 2 token-chunk streams (LDWEIGHTS deduped).
                    for oi in range(OB // P):
                        pms = [
                            psmp.tile([P, TH], F32, name="pm", tag="pm")
                            for _ in range(NTH)
                        ]
                        for k in range(KT):
                            wb = subs[k // WCH]
                            st = wb[:, k % WCH, ts(oi, P)]
                            xk = xt[k // XG]
                            for th in range(NTH):
                                nc.tensor.matmul(
                                    pms[th][:],
                                    st,
                                    xk[:, k % XG, ts(th, TH)],
                                    start=(k == 0),
                                    stop=(k == KT - 1),
                                )
                        zt = zstp.tile([P, T], BF16, name="zt", tag="zt")
                        for th in range(NTH):
                            nc.scalar.copy(zt[:, ts(th, TH)], pms[th][:])
                        nc.sync.dma_start(zs_d.ap()[ts(ob * 2 + oi, P), :], zt[:])

                # x stream starts first (SP queue), w blocks 0/1 prep on the
                # ACT queue in parallel; early x groups are prioritized so the
                # matmul stream never starves while w competes for HBM
                nc.sync.dma_start(xt[0][:], x2_d.ap()[0])
                wb_cur = prep(0, 0)
                for g in range(1, 2):
                    nc.sync.dma_start(xt[g][:], x2_d.ap()[g])
                wb_next = prep(1, 1)
                for g in range(2, NXG):
                    nc.sync.dma_start(xt[g][:], x2_d.ap()[g])

                for ob in range(NOB):
                    mm_block(ob, wb_cur)
                    wb_cur = wb_next
                    wb_next = prep(ob + 2, ob) if ob + 2 < NOB else None
    n = _dedupe_ldweights(nc)
    assert 900 <= n <= NOB * (OB // P) * KT, n
    nc.compile()
    return nc


def _get_nc():
    global _cached_nc
    if _cached_nc is None:
        _cached_nc = _build_program()
    return _cached_nc


def _host_inputs(x, weight):
    """Per-core input dicts (layout/transport prep only):
    x2: bf16(2*x_shard.T) grouped [NXG, P, XG, T];
    wt: fp8(w.T) block-major [NOB, P, KT, OB] (sign-exact), replicated."""
    w8 = weight.T.astype(ml_dtypes.float8_e4m3)  # [K, OUT], sign preserved
    wt = np.ascontiguousarray(
        w8.reshape(KT, P, NOB, OB).transpose(2, 1, 0, 3)
    )
    x2 = (2.0 * x).astype(np.float32)
    in_maps = []
    for c in range(N_CORES):
        x2t = np.ascontiguousarray(
            x2[c * T : (c + 1) * T]
            .T.reshape(NXG, XG, P, T)
            .transpose(0, 2, 1, 3)
        ).astype(ml_dtypes.bfloat16)
        in_maps.append({"x2": x2t, "wt": wt})
    return in_maps


def kernel(x: np.ndarray, weight: np.ndarray, bias: np.ndarray) -> np.ndarray:
    x = np.ascontiguousarray(np.asarray(x, dtype=np.float32))
    weight = np.ascontiguousarray(np.asarray(weight, dtype=np.float32))
    bias = np.ascontiguousarray(np.asarray(bias, dtype=np.float32))
    assert x.shape == (N_TOK, K_IN) and weight.shape == (N_OUT, K_IN)

    nc = _get_nc()
    in_maps = _host_inputs(x, weight)
    res = bass_utils.run_bass_kernel_spmd(nc, in_maps, core_ids=list(range(N_CORES)))
    z = np.empty((N_TOK, N_OUT), dtype=np.float32)
    for c in range(N_CORES):
        np.copyto(z[c * T : (c + 1) * T], res.results[c]["zs"].T.astype(np.float32))
    if np.any(bias):
        z += bias[None, :]
    return z


# ---------------------------------------------------------------------------
# HW timing support (not used by the grading path; test.py calls this).
# The axon PJRT dispatch overhead (~57-110 ms, noisy) swamps a single kernel
# execution, so we measure on-device time with For_i-looped variants:
# slope of min wall time vs loop count. Dispatch noise is additive-positive,
# so the global min per L over all rounds is the robust estimator; three L
# values let us check linearity.
# ---------------------------------------------------------------------------


def _make_runner(nc, n_cores=N_CORES):
    import jax
    from jax.sharding import Mesh, PartitionSpec
    from jax.experimental.shard_map import shard_map
    from concourse import bass2jax

    bass2jax.install_neuronx_cc_hook()
    partition_name = nc.partition_id_tensor.name if nc.partition_id_tensor else None
    in_names, out_names, out_avals, zero_outs = [], [], [], []
    for alloc in nc.m.functions[0].allocations:
        if not isinstance(alloc, mybir.MemoryLocationSet):
            continue
        name = alloc.memorylocations[0].name
        if alloc.kind == "ExternalInput":
            if name != partition_name:
                in_names.append(name)
        elif alloc.kind == "ExternalOutput":
            out_names.append(name)
            out_avals.append(
                jax.core.ShapedArray(tuple(alloc.tensor_shape), mybir.dt.np(alloc.dtype))
            )
            zero_outs.append(
                np.zeros(tuple(alloc.tensor_shape), mybir.dt.np(alloc.dtype))
            )
    n_params, n_outs = len(in_names), len(out_avals)
    all_in_names = list(in_names) + list(out_names)
    if partition_name is not None:
        all_in_names.append(partition_name)

    def _body(*args):
        operands = list(args)
        if partition_name is not None:
            operands.append(bass2jax.partition_id_tensor())
        return tuple(
            bass2jax._bass_exec_p.bind(
                *operands,
                out_avals=tuple(out_avals),
                in_names=tuple(all_in_names),
                out_names=tuple(out_names),
                lowering_input_output_aliases=(),
                sim_require_finite=True,
                sim_require_nnan=True,
                nc=nc,
            )
        )

    donate = tuple(range(n_params, n_params + n_outs))
    devices = jax.devices()[:n_cores]
    mesh = Mesh(np.asarray(devices), ("core",))
    in_specs = (PartitionSpec("core"),) * (n_params + n_outs)
    out_specs = (PartitionSpec("core"),) * n_outs
    jitted = jax.jit(
        shard_map(_body, mesh=mesh, in_specs=in_specs, out_specs=out_specs,
                  check_rep=False),
        donate_argnums=donate,
        keep_unused=True,
    )
    return jitted, in_names, zero_outs


def _min_wall(jitted, ins, zero_outs_global, nrep):
    import time
    import jax

    best = float("inf")
    for _ in range(nrep):
        zo = [jax.device_put(z) for z in zero_outs_global]
        jax.block_until_ready(zo)
        t0 = time.perf_counter()
        outs = jitted(*ins, *zo)
        jax.block_until_ready(outs)
        best = min(best, time.perf_counter() - t0)
    return best


def measure_hw_time_ns(inputs, Ls=(1, 65, 129), nrep=6, rounds=6):
    import jax

    x = np.ascontiguousarray(np.asarray(inputs["x"], dtype=np.float32))
    weight = np.ascontiguousarray(np.asarray(inputs["weight"], dtype=np.float32))
    in_maps = _host_inputs(x, weight)

    runners = []
    for L in Ls:
        nc = _build_program(loop=L)
        jitted, in_names, zero_outs = _make_runner(nc)
        concat_in = [
            np.concatenate(
                [np.asarray(in_maps[c][name]) for c in range(N_CORES)], axis=0
            )
            for name in in_names
        ]
        ins = [jax.device_put(a) for a in concat_in]
        jax.block_until_ready(ins)
        zo_global = [np.concatenate([z] * N_CORES, axis=0) for z in zero_outs]
        # warmup
        outs = jitted(*ins, *[jax.device_put(z) for z in zo_global])
        jax.block_until_ready(outs)
        runners.append((jitted, ins, zo_global))

    t_min = [float("inf")] * len(Ls)
    for r in range(rounds):
        for i in range(len(Ls)):
            jitted, ins, zo = runners[i]
            t_min[i] = min(t_min[i], _min_wall(jitted, ins, zo, nrep))
        msg = " ".join(f"t({L})={t_min[i]*1e3:.2f}ms" for i, L in enumerate(Ls))
        pair = (t_min[-1] - t_min[0]) / (Ls[-1] - Ls[0]) * 1e9
        print(f"  timing round {r}: {msg} slope={pair:.0f}ns")
    # least-squares slope over the L points
    Lv = np.asarray(Ls, dtype=np.float64)
    tv = np.asarray(t_min, dtype=np.float64)
    slope = float(np.cov(Lv, tv, bias=True)[0, 1] / np.var(Lv))
    return slope * 1e9


# revision 14
# speedup vs baseline: 2.0797x; 1.9929x over previous
"""BinaryLinear TRN2 kernel: z = x @ sign(weight).T + bias.

x [8192, 4096] f32, weight [4096, 4096] f32, bias [4096] f32 (zeros).

Strategy (8 NeuronCores, SPMD, data-parallel over the 8192-token batch;
weight replicated; host does layout/transport prep only, all arithmetic
on device):

  - bf16 single-pass matmul. The PE moving feed is the limit: bf16
    streams 1 column/cycle (16384 MAC/cyc); fp8 DoubleRow doubles that
    but needs a two-pass hi/lo split for precision, landing at the same
    ~437 us/core floor (measured 444-454 us on HW for both). bf16
    single-pass hits the floor with half the instructions, no x
    quantization, FWL-eligible 128-col weight loads, ~1.6e-3 rel err.
  - Host ships bf16(2*x_shard.T) and fp8(w.T) (sign-exact transport,
    half the bytes) in BLOCK-MAJOR layouts so every DMA is long
    contiguous runs per partition (2-8 KB) - transposed-layout 512 B
    runs were the previous bottleneck (~90 GB/s effective; the 33.5 MB
    weight stream dominated the kernel).
  - Device binarize: bitcast the fp8 weight to uint8 and take the sign
    bit: (u8 is_lt 128) - 0.5 = sign(w)/2 with sign(-0)=-1 for
    rounded-up tiny negatives and sign(+0)=+1 - exactly matching the
    reference's sign(0)=+1. One DVE/Pool tensor_scalar per sub-chunk.
    z = (sign(w)/2) @ (2x) exactly.
  - PE runs ONLY matmuls: stationary [128,128] bf16 +-0.5 weights,
    moving 512 tokens, psum [128,512] (one bank), 32-deep accumulation,
    stationary shared across the 2 token chunks (redundant LDWEIGHTS
    deleted post-legalization - the PE array keeps its weights).
  - x streams on the SP DMA queue (prioritized at startup), w blocks on
    the ACT queue in 8-plane sub-chunks binarized ahead (2 blocks in
    flight); z written as bf16 zT, host transposes/upcasts on gather.
"""

import numpy as np
import ml_dtypes

import concourse.bacc as bacc
import concourse.bass as bass
import concourse.mybir as mybir
import concourse.tile as tile
from concourse import bass_utils
from concourse.bass import ts

P = 128
N_CORES = 8
N_TOK, K_IN, N_OUT = 8192, 4096, 4096
T = N_TOK // N_CORES  # 1024 tokens per core
KT = K_IN // P  # 32 k-planes
XG = 4  # x planes per DMA group
NXG = KT // XG  # 8 x tiles
OB = 256  # out-feature block width
NOB = N_OUT // OB  # 16 blocks
TH = 512  # token chunk per psum tile
NTH = T // TH  # 2
WCH = 8  # k-planes per w prep sub-chunk
NWCH = KT // WCH  # 4 sub-chunks per block

F32 = mybir.dt.float32
BF16 = mybir.dt.bfloat16
F8 = mybir.dt.float8e4
U8 = mybir.dt.uint8

_cached_nc = None


def _dedupe_ldweights(nc):
    """Remove InstLdweights whose weight AP is identical to the previous
    one with only InstMatmult instructions in between (the PE array still
    holds those weights). Waits on a removed load are hoisted onto the
    next PE instruction."""
    n_removed = 0
    for blk in nc.m.functions[0].blocks:
        insts = blk.instructions
        keep = []
        last_ld_key = None
        pending_waits = []
        for inst in insts:
            if inst.engine != mybir.EngineType.PE:
                keep.append(inst)
                continue
            if isinstance(inst, mybir.InstLdweights):
                key = str(inst.ins[0]) + f"|{inst.perf_mode}"
                if key == last_ld_key:
                    si = inst.sync_info
                    if si is not None:
                        pending_waits.extend(si.on_wait)
                        assert not si.on_update, "dedupe: LD carries updates"
                    n_removed += 1
                    continue
                last_ld_key = key
            elif not isinstance(inst, mybir.InstMatmult):
                last_ld_key = None
            if pending_waits:
                si = inst.sync_info
                if si is None:
                    inst.sync_info = mybir.SyncInfo(
                        on_wait=pending_waits, on_update=[]
                    )
                else:
                    si.on_wait = list(si.on_wait) + pending_waits
                pending_waits = []
            keep.append(inst)
        assert not pending_waits
        blk.instructions = keep
    return n_removed


def _build_program(loop: int = 0):
    """loop=0: plain kernel. loop=L>0: body wrapped in an on-device For_i
    (used for HW timing via the slope method)."""
    nc = bacc.Bacc("TRN2", target_bir_lowering=False, debug=False)
    # bf16(2*x) shard: [group, kp, plane-in-group, tok] (8 KB runs)
    x2_d = nc.dram_tensor("x2", [NXG, P, XG, T], BF16, kind="ExternalInput")
    # fp8(w.T), block-major: [block, kp, plane, out-in-block] (8 KB runs)
    w_d = nc.dram_tensor("wt", [NOB, P, KT, OB], F8, kind="ExternalInput")
    zs_d = nc.dram_tensor("zs", [N_OUT, T], BF16, kind="ExternalOutput")

    import contextlib

    IS_LT = mybir.AluOpType.is_lt
    SUB = mybir.AluOpType.subtract

    with tile.TileContext(nc) as tc:
        with (
            tc.tile_pool(name="xt", bufs=1) as xtp,
            tc.tile_pool(name="wraw", bufs=8) as wrawp,
            tc.tile_pool(name="wbin", bufs=3) as wbinp,
            tc.tile_pool(name="zst", bufs=3) as zstp,
            tc.tile_pool(name="psm", bufs=6, space="PSUM") as psmp,
        ):
            # resident bf16 x tiles, one per XG-plane group
            xt = [
                xtp.tile([P, XG, T], BF16, name=f"xt{i}") for i in range(NXG)
            ]

            loop_cm = tc.For_i(0, loop, 1) if loop else contextlib.nullcontext()
            with loop_cm:
                # ---- Weight block prep (ACT hwdge queue): DMA fp8 wT
                # sub-chunks (contiguous 2 KB runs), binarize via the uint8
                # sign bit to +-0.5 bf16 on DVE/Pool ----
                def prep(ob, eng_off=0):
                    subs = []
                    for s in range(NWCH):
                        wr = wrawp.tile([P, WCH, OB], F8, name="wr", tag="wr")
                        nc.scalar.dma_start(
                            wr[:], w_d.ap()[ob, :, s * WCH : (s + 1) * WCH, :]
                        )
                        wb = wbinp.tile([P, WCH, OB], BF16, name=f"wb{s}", tag=f"wb{s}")
                        # all binarize on DVE: gpsimd tensor ops measure ~25x
                        # slower than the cost model claims and gate the MMs
                        nc.vector.tensor_scalar(
                            wb[:], wr[:].bitcast(U8), 128.0, 0.5, IS_LT, SUB
                        )
                        subs.append(wb)
                    return subs

                def mm_block(ob, subs):
                    # zT orientation: psum tiles [128 out, 512 tok];
                    # stationary = bf16 +-0.5 weight plane column, shared
                    # by the 2 token-chunk streams (LDWEIGHTS deduped).
                    for oi in range(OB // P):
                        pms = [
                            psmp.tile([P, TH], F32, name="pm", tag="pm")
                            for _ in range(NTH)
                        ]
                        for k in range(KT):
                            wb = subs[k // WCH]
                            st = wb[:, k % WCH, ts(oi, P)]
                            xk = xt[k // XG]
                            for th in range(NTH):
                                nc.tensor.matmul(
                                    pms[th][:],
                                    st,
                                    xk[:, k % XG, ts(th, TH)],
                                    start=(k == 0),
                                    stop=(k == KT - 1),
                                )
                        zt = zstp.tile([P, T], BF16, name="zt", tag="zt")
                        for th in range(NTH):
                            nc.scalar.copy(zt[:, ts(th, TH)], pms[th][:])
                        nc.sync.dma_start(zs_d.ap()[ts(ob * 2 + oi, P), :], zt[:])

                # x stream starts first (SP queue), w blocks 0/1 prep on the
                # ACT queue in parallel; early x groups are prioritized so the
                # matmul stream never starves while w competes for HBM
                nc.sync.dma_start(xt[0][:], x2_d.ap()[0])
                wb_cur = prep(0, 0)
                for g in range(1, 2):
                    nc.sync.dma_start(xt[g][:], x2_d.ap()[g])
                wb_next = prep(1, 1)
                for g in range(2, NXG):
                    nc.sync.dma_start(xt[g][:], x2_d.ap()[g])

                for ob in range(NOB):
                    mm_block(ob, wb_cur)
                    wb_cur = wb_next
                    wb_next = prep(ob + 2, ob) if ob + 2 < NOB else None
    n = _dedupe_ldweights(nc)
    assert 900 <= n <= NOB * (OB // P) * KT, n
    nc.compile()
    return nc


def _get_nc():
    global _cached_nc
    if _cached_nc is None:
        _cached_nc = _build_program()
    return _cached_nc


def _host_inputs(x, weight):
    """Per-core input dicts (layout/transport prep only):
    x2: bf16(2*x_shard.T) grouped [NXG, P, XG, T];
    wt: fp8(w.T) block-major [NOB, P, KT, OB] (sign-exact), replicated."""
    w8 = weight.T.astype(ml_dtypes.float8_e4m3)  # [K, OUT], sign preserved
    wt = np.ascontiguousarray(
        w8.reshape(KT, P, NOB, OB).transpose(2, 1, 0, 3)
    )
    x2 = (2.0 * x).astype(np.float32)
    in_maps = []
    for c in range(N_CORES):
        x2t = np.ascontiguousarray(
            x2[c * T : (c + 1) * T]
            .T.reshape(NXG, XG, P, T)
            .transpose(0, 2, 1, 3)
        ).astype(ml_dtypes.bfloat16)
        in_maps.append({"x2": x2t, "wt": wt})
    return in_maps


def kernel(x: np.ndarray, weight: np.ndarray, bias: np.ndarray) -> np.ndarray:
    x = np.ascontiguousarray(np.asarray(x, dtype=np.float32))
    weight = np.ascontiguousarray(np.asarray(weight, dtype=np.float32))
    bias = np.ascontiguousarray(np.asarray(bias, dtype=np.float32))
    assert x.shape == (N_TOK, K_IN) and weight.shape == (N_OUT, K_IN)

    nc = _get_nc()
    in_maps = _host_inputs(x, weight)
    res = bass_utils.run_bass_kernel_spmd(nc, in_maps, core_ids=list(range(N_CORES)))
    z = np.empty((N_TOK, N_OUT), dtype=np.float32)
    for c in range(N_CORES):
        np.copyto(z[c * T : (c + 1) * T], res.results[c]["zs"].T.astype(np.float32))
    if np.any(bias):
        z += bias[None, :]
    return z


# ---------------------------------------------------------------------------
# HW timing support (not used by the grading path; test.py calls this).
# The axon PJRT dispatch overhead (~57-110 ms, noisy) swamps a single kernel
# execution, so we measure on-device time with For_i-looped variants:
# slope of min wall time vs loop count. Dispatch noise is additive-positive,
# so the global min per L over all rounds is the robust estimator; three L
# values let us check linearity.
# ---------------------------------------------------------------------------


def _make_runner(nc, n_cores=N_CORES):
    import jax
    from jax.sharding import Mesh, PartitionSpec
    from jax.experimental.shard_map import shard_map
    from concourse import bass2jax

    bass2jax.install_neuronx_cc_hook()
    partition_name = nc.partition_id_tensor.name if nc.partition_id_tensor else None
    in_names, out_names, out_avals, zero_outs = [], [], [], []
    for alloc in nc.m.functions[0].allocations:
        if not isinstance(alloc, mybir.MemoryLocationSet):
            continue
        name = alloc.memorylocations[0].name
        if alloc.kind == "ExternalInput":
            if name != partition_name:
                in_names.append(name)
        elif alloc.kind == "ExternalOutput":
            out_names.append(name)
            out_avals.append(
                jax.core.ShapedArray(tuple(alloc.tensor_shape), mybir.dt.np(alloc.dtype))
            )
            zero_outs.append(
                np.zeros(tuple(alloc.tensor_shape), mybir.dt.np(alloc.dtype))
            )
    n_params, n_outs = len(in_names), len(out_avals)
    all_in_names = list(in_names) + list(out_names)
    if partition_name is not None:
        all_in_names.append(partition_name)

    def _body(*args):
        operands = list(args)
        if partition_name is not None:
            operands.append(bass2jax.partition_id_tensor())
        return tuple(
            bass2jax._bass_exec_p.bind(
                *operands,
                out_avals=tuple(out_avals),
                in_names=tuple(all_in_names),
                out_names=tuple(out_names),
                lowering_input_output_aliases=(),
                sim_require_finite=True,
                sim_require_nnan=True,
                nc=nc,
            )
        )

    donate = tuple(range(n_params, n_params + n_outs))
    devices = jax.devices()[:n_cores]
    mesh = Mesh(np.asarray(devices), ("core",))
    in_specs = (PartitionSpec("core"),) * (n_params + n_outs)
    out_specs = (PartitionSpec("core"),) * n_outs
    jitted = jax.jit(
        shard_map(_body, mesh=mesh, in_specs=in_specs, out_specs=out_specs,
                  check_rep=False),
        donate_argnums=donate,
        keep_unused=True,
    )
    return jitted, in_names, zero_outs


def _min_wall(jitted, ins, zero_outs_global, nrep):
    import time
    import jax

    best = float("inf")
    for _ in range(nrep):
        zo = [jax.device_put(z) for z in zero_outs_global]
        jax.block_until_ready(zo)
        t0 = time.perf_counter()
        outs = jitted(*ins, *zo)
        jax.block_until_ready(outs)
        best = min(best, time.perf_counter() - t0)
    return best


def measure_hw_time_ns(inputs, Ls=(1, 65, 129), nrep=6, rounds=6):
    import jax

    x = np.ascontiguousarray(np.asarray(inputs["x"], dtype=np.float32))
    weight = np.ascontiguousarray(np.asarray(inputs["weight"], dtype=np.float32))
    in_maps = _host_inputs(x, weight)

    runners = []
    for L in Ls:
        nc = _build_program(loop=L)
        jitted, in_names, zero_outs = _make_runner(nc)
        concat_in = [
            np.concatenate(
                [np.asarray(in_maps[c][name]) for c in range(N_CORES)], axis=0
            )
            for name in in_names
        ]
        ins = [jax.device_put(a) for a in concat_in]
        jax.block_until_ready(ins)
        zo_global = [np.concatenate([z] * N_CORES, axis=0) for z in zero_outs]
        # warmup
        outs = jitted(*ins, *[jax.device_put(z) for z in zo_global])
        jax.block_until_ready(outs)
        runners.append((jitted, ins, zo_global))

    t_min = [float("inf")] * len(Ls)
    for r in range(rounds):
        for i in range(len(Ls)):
            jitted, ins, zo = runners[i]
            t_min[i] = min(t_min[i], _min_wall(jitted, ins, zo, nrep))
        msg = " ".join(f"t({L})={t_min[i]*1e3:.2f}ms" for i, L in enumerate(Ls))
        pair = (t_min[-1] - t_min[0]) / (Ls[-1] - Ls[0]) * 1e9
        print(f"  timing round {r}: {msg} slope={pair:.0f}ns")
    # least-squares slope over the L points
    Lv = np.asarray(Ls, dtype=np.float64)
    tv = np.asarray(t_min, dtype=np.float64)
    slope = float(np.cov(Lv, tv, bias=True)[0, 1] / np.var(Lv))
    return slope * 1e9
